# revision 3
# baseline (speedup 1.0000x reference)
"""Trainium2 Bass kernel for NayheinMiniAttention (16-head causal attention
with RoPE, B=2, S=2048, hidden=2048, fp32).

Sharding: 8 cores = 2 batches x 4 head-groups (4 heads each).
Per core (batch b, heads hg*4..hg*4+3):
  - Q/K projections emit QT/KT in [d, s] layout (W-col stationary, xT moving),
    RoPE applied from precomputed transposed cos/sin tables.
  - V projection emits V in natural [s, d] layout (xT-block stationary,
    Wv-row moving).
  - Attention computed in the [k, q] orientation: scoresT = KT_blk.T @ QT_blk,
    exp on ScalarE (no max subtraction needed: |scores| <= ~6), causal mask
    via affine_select on the diagonal blocks, softmax denominator via a
    ones-matrix matmul accumulated in PSUM, normalization by reciprocal
    broadcast, P@V accumulated directly in the [d, q] layout.
  - Output projection y = AOT.T @ WoT gives a partial [s, 2048] output;
    host sums the 4 head-group partials per batch.

Matmul dtypes: float32r (fp32 bits, ~13-bit-mantissa PE mode, 1 cycle/row,
4x faster than plain fp32) for projections and output; bf16 for the
attention inner matmuls (score/PV operands are stored bf16 to fit SBUF).
"""

import os
import sys
import math

sys.path.insert(0, "/opt/trn_rl_repo")

import numpy as np
import concourse.bass as bass
import concourse.mybir as mybir
import concourse.tile as tile
from concourse import bacc
from concourse.bass_utils import run_bass_kernel_spmd

DT = mybir.dt

B = 2
S = 2048
H = 2048
NH = 16
HD = 128
ROPE_THETA = 10000.0

P = 128
NHG = 4  # head groups (cores per batch)
HPC = 4  # heads per core
OC = HPC * HD  # per-core projection width (512)
KT = H // P  # 16 contraction tiles
SQ = 4  # s-quarters
SBLK = S // SQ  # 512
NST = S // P  # 16 s-tiles

_CACHE = {}


def _build_nc():
    nc = bacc.Bacc("TRN2", target_bir_lowering=False, debug=False, num_devices=8)

    x_d = nc.dram_tensor("x", [SQ, P, KT, SBLK], DT.float32r, kind="ExternalInput")
    wq_d = nc.dram_tensor("wq", [HPC, P, KT, P], DT.float32r, kind="ExternalInput")
    wk_d = nc.dram_tensor("wk", [HPC, P, KT, P], DT.float32r, kind="ExternalInput")
    wv_d = nc.dram_tensor("wv", [KT, P, OC], DT.float32r, kind="ExternalInput")
    wo_d = nc.dram_tensor("wo", [P, HPC, H], DT.float32r, kind="ExternalInput")
    cos_d = nc.dram_tensor("cos", [64, S], DT.float32, kind="ExternalInput")
    sin_d = nc.dram_tensor("sin", [64, S], DT.float32, kind="ExternalInput")
    y_d = nc.dram_tensor("y", [S, H], DT.float32, kind="ExternalOutput")

    with tile.TileContext(nc) as tc:
        with (
            tc.tile_pool(name="const", bufs=1) as cpool,
            tc.tile_pool(name="xq", bufs=1) as xpool,
            tc.tile_pool(name="wo", bufs=1) as wopool,
            tc.tile_pool(name="wcol", bufs=2) as wpool,
            tc.tile_pool(name="wvrow", bufs=2) as wvpool,
            tc.tile_pool(name="qk", bufs=1) as qkpool,
            tc.tile_pool(name="vsb", bufs=1) as vpool,
            tc.tile_pool(name="rope", bufs=2) as rpool,
            tc.tile_pool(name="expt", bufs=2) as epool,
            tc.tile_pool(name="aot", bufs=2) as aotpool,
            tc.tile_pool(name="bcast", bufs=2) as bpool,
            tc.tile_pool(name="ysb", bufs=3) as ypool,
            tc.tile_pool(name="pmm", bufs=2, space="PSUM") as pmm,
            tc.tile_pool(name="pst", bufs=2, space="PSUM") as pst,
            tc.tile_pool(name="pb", bufs=2, space="PSUM") as pb,
            tc.tile_pool(name="paot", bufs=2, space="PSUM") as paot,
        ):
            # constants
            cos_sb = cpool.tile([64, S], DT.float32, tag="cos")
            sin_sb = cpool.tile([64, S], DT.float32, tag="sin")
            nc.sync.dma_start(out=cos_sb[:], in_=cos_d[:])
            nc.sync.dma_start(out=sin_sb[:], in_=sin_d[:])
            ones128 = cpool.tile([P, P], DT.bfloat16, tag="ones")
            nc.vector.memset(ones128[:], 1.0)

            wo_sb = wopool.tile([P, HPC, H], DT.float32r, tag="wo")
            nc.sync.dma_start(out=wo_sb[:], in_=wo_d[:])

            qt_sb = qkpool.tile([P, HPC, S], DT.bfloat16, tag="qt")
            kt_sb = qkpool.tile([P, HPC, S], DT.bfloat16, tag="kt")
            v_sb = vpool.tile([P, NST, OC], DT.bfloat16, tag="v")

            for sq in range(SQ):
                # ---- projections for s-quarter sq ----
                x_chunk = xpool.tile([P, KT, SBLK], DT.float32r, tag="xq")
                nc.sync.dma_start(out=x_chunk[:], in_=x_d[sq])

                for pi, (w_dram, out_sb) in enumerate(
                    [(wq_d, qt_sb), (wk_d, kt_sb)]
                ):
                    for t in range(HPC):
                        w_col = wpool.tile([P, KT, P], DT.float32r, tag="wcol")
                        nc.sync.dma_start(out=w_col[:], in_=w_dram[t])
                        pq = pmm.tile([P, SBLK], DT.float32, tag="mm")
                        for kt in range(KT):
                            nc.tensor.matmul(
                                pq[:],
                                w_col[:, kt, :],
                                x_chunk[:, kt, :],
                                start=(kt == 0),
                                stop=(kt == KT - 1),
                            )
                        # RoPE (tables are [64, S]; rows repeat across halves):
                        #   out[0:64]   = pq[0:64]*c - pq[64:128]*s
                        #   out[64:128] = pq[64:128]*c + pq[0:64]*s
                        c_blk = cos_sb[:, sq * SBLK : (sq + 1) * SBLK]
                        s_blk = sin_sb[:, sq * SBLK : (sq + 1) * SBLK]
                        t1 = rpool.tile([P, SBLK], DT.float32, tag="t1")
                        t2 = rpool.tile([P, SBLK], DT.float32, tag="t2")
                        ob = out_sb[:, t, sq * SBLK : (sq + 1) * SBLK]
                        nc.vector.tensor_mul(t1[0:64, :], pq[0:64, :], c_blk)
                        nc.vector.tensor_mul(t1[64:128, :], pq[64:128, :], c_blk)
                        nc.vector.tensor_mul(t2[0:64, :], pq[64:128, :], s_blk)
                        nc.vector.tensor_mul(t2[64:128, :], pq[0:64, :], s_blk)
                        nc.vector.tensor_sub(ob[0:64, :], t1[0:64, :], t2[0:64, :])
                        nc.vector.tensor_add(
                            ob[64:128, :], t1[64:128, :], t2[64:128, :]
                        )

                # ---- V for this quarter (natural [s, o] layout) ----
                for vp in range(2):
                    psv0 = pmm.tile([P, SBLK], DT.float32, tag="mm")
                    psv1 = pmm.tile([P, SBLK], DT.float32, tag="mm")
                    psv = [psv0, psv1]
                    for kt in range(KT):
                        wv_row = wvpool.tile([P, OC], DT.float32r, tag="wvrow")
                        nc.sync.dma_start(out=wv_row[:], in_=wv_d[kt])
                        for i2 in range(2):
                            st_loc = vp * 2 + i2
                            nc.tensor.matmul(
                                psv[i2][:],
                                x_chunk[:, kt, st_loc * P : (st_loc + 1) * P],
                                wv_row[:],
                                start=(kt == 0),
                                stop=(kt == KT - 1),
                            )
                    for i2 in range(2):
                        st_glob = sq * 4 + vp * 2 + i2
                        nc.scalar.copy(v_sb[:, st_glob, :], psv[i2][:])

                # ---- attention for q-group g = sq ----
                g = sq
                aotg = aotpool.tile([P, HPC, SBLK], DT.float32r, tag="aot")
                for h in range(HPC):
                    psb = pb.tile([P, SBLK], DT.float32, tag="b")
                    psaot = paot.tile([P, SBLK], DT.float32, tag="a")
                    jmax = 4 * g + 3
                    for j in range(jmax + 1):
                        stt = pst.tile([P, SBLK], DT.float32, tag="st")
                        nc.tensor.matmul(
                            stt[:],
                            kt_sb[:, h, j * P : (j + 1) * P],
                            qt_sb[:, h, g * SBLK : (g + 1) * SBLK],
                            start=True,
                            stop=True,
                        )
                        expt = epool.tile([P, SBLK], DT.bfloat16, tag="e")
                        nc.scalar.activation(
                            expt[:], stt[:], mybir.ActivationFunctionType.Exp
                        )
                        if j >= 4 * g:
                            nc.gpsimd.affine_select(
                                out=expt[:],
                                in_=expt[:],
                                compare_op=mybir.AluOpType.is_ge,
                                fill=0.0,
                                base=(4 * g - j) * P,
                                channel_multiplier=-1,
                                pattern=[[1, SBLK]],
                            )
                        nc.tensor.matmul(
                            psb[:],
                            ones128[:],
                            expt[:],
                            start=(j == 0),
                            stop=(j == jmax),
                        )
                        nc.tensor.matmul(
                            psaot[:],
                            v_sb[:, j, h * HD : (h + 1) * HD],
                            expt[:],
                            start=(j == 0),
                            stop=(j == jmax),
                        )
                    bc = bpool.tile([P, SBLK], DT.float32, tag="bc")
                    nc.vector.reciprocal(bc[:], psb[:])
                    nc.vector.tensor_mul(aotg[:, h, :], psaot[:], bc[:])

                # ---- output projection for s-tiles of group g ----
                for il in range(4):
                    srow = (g * 4 + il) * P
                    for mb in range(4):
                        pym = pmm.tile([P, SBLK], DT.float32, tag="mm")
                        for h in range(HPC):
                            nc.tensor.matmul(
                                pym[:],
                                aotg[:, h, il * P : (il + 1) * P],
                                wo_sb[:, h, mb * SBLK : (mb + 1) * SBLK],
                                start=(h == 0),
                                stop=(h == HPC - 1),
                            )
                        y_sb = ypool.tile([P, SBLK], DT.float32, tag="y")
                        nc.vector.tensor_copy(y_sb[:], pym[:])
                        nc.sync.dma_start(
                            out=y_d[srow : srow + P, mb * SBLK : (mb + 1) * SBLK],
                            in_=y_sb[:],
                        )

    nc.compile()
    return nc


def _pack_inputs(hidden_states, Wq, Wk, Wv, Wo):
    """Per-core input dicts. Core c = b*4 + hg."""
    scale = 1.0 / math.sqrt(HD)

    # RoPE tables, transposed layout [d, s], sign folded into sin.
    inv_freq = (1.0 / (ROPE_THETA ** (np.arange(0, HD, 2) / HD))).astype(np.float64)
    freqs = np.arange(S, dtype=np.float64)[:, None] * inv_freq[None, :]  # [S, 64]
    cos_h = np.ascontiguousarray(np.cos(freqs).T.astype(np.float32))  # [64, S]
    sin_h = np.ascontiguousarray(np.sin(freqs).T.astype(np.float32))  # [64, S]

    in_maps = []
    for c in range(8):
        b, hg = c // NHG, c % NHG
        hs = np.ascontiguousarray(hidden_states[b])  # [S, H]
        x_packed = np.ascontiguousarray(
            hs.reshape(SQ, SBLK, KT, P).transpose(0, 3, 2, 1)
        )  # [sq, Ph, kt, s]

        def w_cols(Wmat, sc=1.0):
            A = (Wmat[hg * OC : (hg + 1) * OC, :] * sc).astype(np.float32)  # [o, h]
            return np.ascontiguousarray(
                A.T.reshape(KT, P, HPC, P).transpose(2, 1, 0, 3)
            )  # [t, Ph, kt, o]

        wq_p = w_cols(Wq, scale)
        wk_p = w_cols(Wk)
        wv_p = np.ascontiguousarray(
            Wv[hg * OC : (hg + 1) * OC, :].T.reshape(KT, P, OC)
        )  # [kt, Ph, o]
        wo_p = np.ascontiguousarray(
            Wo[:, hg * OC : (hg + 1) * OC].T.reshape(HPC, P, H).transpose(1, 0, 2)
        )  # [Po, h, m]

        in_maps.append(
            {
                "x": x_packed,
                "wq": wq_p,
                "wk": wk_p,
                "wv": wv_p,
                "wo": wo_p,
                "cos": cos_h,
                "sin": sin_h,
            }
        )
    return in_maps


def _get_nc():
    if "nc" not in _CACHE:
        _CACHE["nc"] = _build_nc()
    return _CACHE["nc"]


def kernel(hidden_states, Wq, Wk, Wv, Wo, attention_mask=None, **_ignored):
    hidden_states = np.asarray(hidden_states, dtype=np.float32)
    Wq = np.asarray(Wq, dtype=np.float32)
    Wk = np.asarray(Wk, dtype=np.float32)
    Wv = np.asarray(Wv, dtype=np.float32)
    Wo = np.asarray(Wo, dtype=np.float32)

    nc = _get_nc()
    in_maps = _pack_inputs(hidden_states, Wq, Wk, Wv, Wo)

    trace = bool(os.environ.get("KERNEL_TRACE"))
    kwargs = {}
    if trace:
        import types

        try:
            import antenv.axon_hooks  # noqa: F401
        except ImportError:
            from trn_agent_boot.trn_boot import _ntff_profile_via_ctypes

            hook = _ntff_profile_via_ctypes("/opt/axon/libaxon_pjrt.so")
            m = types.ModuleType("antenv.axon_hooks")
            m.get_axon_ntff_profile_hook = lambda: hook
            sys.modules["antenv.axon_hooks"] = m
        from concourse import bass_utils as _bu

        _bu.upload_artifacts = lambda tmpdir: "local://" + tmpdir
        kwargs["trace"] = True

    res = run_bass_kernel_spmd(nc, in_maps, list(range(8)), **kwargs)
    _CACHE["last_exec_time_ns"] = res.exec_time_ns

    out = np.empty((B, S, H), dtype=np.float32)
    for b in range(B):
        acc = res.results[b * NHG + 0]["y"].astype(np.float32)
        for hg in range(1, NHG):
            acc = acc + res.results[b * NHG + hg]["y"]
        out[b] = acc
    return out


# revision 7
# speedup vs baseline: 1.4035x; 1.4035x over previous
"""Trainium2 Bass kernel for NayheinMiniAttention (16-head causal attention
with RoPE, B=2, S=2048, hidden=2048, fp32).

Sharding: 8 cores = 2 batches x 4 head-groups (4 heads each).
Per core (batch b, heads hg*4..hg*4+3):
  - Q/K projections emit QT/KT in [d, s] layout (W-col stationary, xT moving),
    RoPE applied from precomputed transposed cos/sin tables.
  - V projection emits V in natural [s, d] layout (xT-block stationary,
    Wv-row moving).
  - Attention computed in the [k, q] orientation: scoresT = KT_blk.T @ QT_blk,
    exp on ScalarE (no max subtraction needed: |scores| <= ~6), causal mask
    via affine_select on the diagonal blocks, softmax denominator via a
    ones-matrix matmul accumulated in PSUM, normalization by reciprocal
    broadcast, P@V accumulated directly in the [d, q] layout.
  - Output projection y = AOT.T @ WoT gives a partial [s, 2048] output;
    host sums the 4 head-group partials per batch.

Matmul dtypes: float32r (fp32 bits, ~13-bit-mantissa PE mode, 1 cycle/row,
4x faster than plain fp32) for projections and output; bf16 for the
attention inner matmuls (score/PV operands are stored bf16 to fit SBUF).
"""

import os
import sys
import math

sys.path.insert(0, "/opt/trn_rl_repo")

import ml_dtypes
import numpy as np
import concourse.bass as bass
import concourse.mybir as mybir
import concourse.tile as tile
from concourse import bacc
from concourse.bass_utils import run_bass_kernel_spmd

DT = mybir.dt

B = 2
S = 2048
H = 2048
NH = 16
HD = 128
ROPE_THETA = 10000.0

P = 128
NHG = 4  # head groups (cores per batch)
HPC = 4  # heads per core
OC = HPC * HD  # per-core projection width (512)
KT = H // P  # 16 contraction tiles
SQ = 4  # s-quarters
SBLK = S // SQ  # 512
NST = S // P  # 16 s-tiles

_CACHE = {}

# matmul operand dtype for the projection / output stages:
#   "bf16"  - fast weight load, halved DMA/SBUF, ~2x rel-err vs f32r
#   "f32r"  - tf32-like PE mode, best accuracy at same matmul rate (but
#             4-byte weight loads keep the PE clock-gate cold)
WDT_NAME = os.environ.get("KERNEL_WDT", "bf16")
WDT = {"bf16": DT.bfloat16, "f32r": DT.float32r}[WDT_NAME]


def _build_nc():
    nc = bacc.Bacc("TRN2", target_bir_lowering=False, debug=False, num_devices=8)

    x_d = nc.dram_tensor("x", [SQ, P, KT, SBLK], WDT, kind="ExternalInput")
    wq_d = nc.dram_tensor("wq", [HPC, P, KT, P], WDT, kind="ExternalInput")
    wk_d = nc.dram_tensor("wk", [HPC, P, KT, P], WDT, kind="ExternalInput")
    wv_d = nc.dram_tensor("wv", [KT, P, OC], WDT, kind="ExternalInput")
    wo_d = nc.dram_tensor("wo", [P, HPC, H], WDT, kind="ExternalInput")
    cos_d = nc.dram_tensor("cos", [64, S], DT.float32, kind="ExternalInput")
    sin_d = nc.dram_tensor("sin", [64, S], DT.float32, kind="ExternalInput")
    y_d = nc.dram_tensor("y", [S, H], DT.float32, kind="ExternalOutput")

    with tile.TileContext(nc) as tc:
        with (
            tc.tile_pool(name="const", bufs=1) as cpool,
            tc.tile_pool(name="xq", bufs=2) as xpool,
            tc.tile_pool(name="wo", bufs=1) as wopool,
            tc.tile_pool(name="wcol", bufs=3) as wpool,
            tc.tile_pool(name="wvrow", bufs=4) as wvpool,
            tc.tile_pool(name="qk", bufs=1) as qkpool,
            tc.tile_pool(name="vsb", bufs=1) as vpool,
            tc.tile_pool(name="rope", bufs=2) as rpool,
            tc.tile_pool(name="expt", bufs=2) as epool,
            tc.tile_pool(name="aot", bufs=2) as aotpool,
            tc.tile_pool(name="bcast", bufs=2) as bpool,
            tc.tile_pool(name="ysb", bufs=3) as ypool,
            tc.tile_pool(name="pmm", bufs=2, space="PSUM") as pmm,
            tc.tile_pool(name="pst", bufs=2, space="PSUM") as pst,
            tc.tile_pool(name="pb", bufs=2, space="PSUM") as pb,
            tc.tile_pool(name="paot", bufs=2, space="PSUM") as paot,
        ):
            # constants
            cos_sb = cpool.tile([64, S], DT.float32, tag="cos")
            sin_sb = cpool.tile([64, S], DT.float32, tag="sin")
            nc.sync.dma_start(out=cos_sb[:], in_=cos_d[:])
            nc.sync.dma_start(out=sin_sb[:], in_=sin_d[:])
            ones128 = cpool.tile([P, P], DT.bfloat16, tag="ones")
            nc.vector.memset(ones128[:], 1.0)

            wo_sb = wopool.tile([P, HPC, H], WDT, tag="wo")
            nc.sync.dma_start(out=wo_sb[:], in_=wo_d[:])

            qt_sb = qkpool.tile([P, HPC, S], DT.bfloat16, tag="qt")
            kt_sb = qkpool.tile([P, HPC, S], DT.bfloat16, tag="kt")
            v_sb = vpool.tile([P, NST, OC], DT.bfloat16, tag="v")

            def rope(pq, sq, ob):
                # RoPE (tables are [64, S]; rows repeat across halves):
                #   out[0:64]   = pq[0:64]*c - pq[64:128]*s
                #   out[64:128] = pq[64:128]*c + pq[0:64]*s
                c_blk = cos_sb[:, sq * SBLK : (sq + 1) * SBLK]
                s_blk = sin_sb[:, sq * SBLK : (sq + 1) * SBLK]
                t1 = rpool.tile([P, SBLK], DT.float32, tag="t1")
                t2 = rpool.tile([P, SBLK], DT.float32, tag="t2")
                nc.vector.tensor_mul(t1[0:64, :], pq[0:64, :], c_blk)
                nc.vector.tensor_mul(t1[64:128, :], pq[64:128, :], c_blk)
                nc.vector.tensor_mul(t2[0:64, :], pq[64:128, :], s_blk)
                nc.vector.tensor_mul(t2[64:128, :], pq[0:64, :], s_blk)
                nc.vector.tensor_sub(ob[0:64, :], t1[0:64, :], t2[0:64, :])
                nc.vector.tensor_add(ob[64:128, :], t1[64:128, :], t2[64:128, :])

            def do_qk(sq, x_chunk, w_dram, out_sb):
                for t in range(HPC):
                    w_col = wpool.tile([P, KT, P], WDT, tag="wcol")
                    nc.sync.dma_start(out=w_col[:], in_=w_dram[t])
                    pq = pmm.tile([P, SBLK], DT.float32, tag="mm")
                    for kt in range(KT):
                        nc.tensor.matmul(
                            pq[:],
                            w_col[:, kt, :],
                            x_chunk[:, kt, :],
                            start=(kt == 0),
                            stop=(kt == KT - 1),
                        )
                    rope(pq, sq, out_sb[:, t, sq * SBLK : (sq + 1) * SBLK])

            def do_v(sq, x_chunk):
                for vp in range(2):
                    psv0 = pmm.tile([P, SBLK], DT.float32, tag="mm")
                    psv1 = pmm.tile([P, SBLK], DT.float32, tag="mm")
                    psv = [psv0, psv1]
                    for kt in range(KT):
                        wv_row = wvpool.tile([P, OC], WDT, tag="wvrow")
                        nc.sync.dma_start(out=wv_row[:], in_=wv_d[kt])
                        for i2 in range(2):
                            st_loc = vp * 2 + i2
                            nc.tensor.matmul(
                                psv[i2][:],
                                x_chunk[:, kt, st_loc * P : (st_loc + 1) * P],
                                wv_row[:],
                                start=(kt == 0),
                                stop=(kt == KT - 1),
                            )
                    for i2 in range(2):
                        st_glob = sq * 4 + vp * 2 + i2
                        nc.scalar.copy(v_sb[:, st_glob, :], psv[i2][:])

            def do_attn_head(g, h, aotg):
                psb = pb.tile([P, SBLK], DT.float32, tag="b")
                psaot = paot.tile([P, SBLK], DT.float32, tag="a")
                jmax = 4 * g + 3
                for j in range(jmax + 1):
                    stt = pst.tile([P, SBLK], DT.float32, tag="st")
                    nc.tensor.matmul(
                        stt[:],
                        kt_sb[:, h, j * P : (j + 1) * P],
                        qt_sb[:, h, g * SBLK : (g + 1) * SBLK],
                        start=True,
                        stop=True,
                    )
                    expt = epool.tile([P, SBLK], DT.bfloat16, tag="e")
                    nc.scalar.activation(
                        expt[:], stt[:], mybir.ActivationFunctionType.Exp
                    )
                    if j >= 4 * g:
                        nc.gpsimd.affine_select(
                            out=expt[:],
                            in_=expt[:],
                            compare_op=mybir.AluOpType.is_ge,
                            fill=0.0,
                            base=(4 * g - j) * P,
                            channel_multiplier=-1,
                            pattern=[[1, SBLK]],
                        )
                    nc.tensor.matmul(
                        psb[:], ones128[:], expt[:],
                        start=(j == 0), stop=(j == jmax),
                    )
                    nc.tensor.matmul(
                        psaot[:],
                        v_sb[:, j, h * HD : (h + 1) * HD],
                        expt[:],
                        start=(j == 0), stop=(j == jmax),
                    )
                bc = bpool.tile([P, SBLK], DT.float32, tag="bc")
                nc.vector.reciprocal(bc[:], psb[:])
                nc.vector.tensor_mul(aotg[:, h, :], psaot[:], bc[:])

            def do_y(g, aotg):
                for il in range(4):
                    srow = (g * 4 + il) * P
                    for mp in range(2):  # m-block pairs: 2 matmuls per LDW
                        pym0 = pmm.tile([P, SBLK], DT.float32, tag="mm")
                        pym1 = pmm.tile([P, SBLK], DT.float32, tag="mm")
                        pyms = [pym0, pym1]
                        for h in range(HPC):
                            for mi in range(2):
                                mb = mp * 2 + mi
                                nc.tensor.matmul(
                                    pyms[mi][:],
                                    aotg[:, h, il * P : (il + 1) * P],
                                    wo_sb[:, h, mb * SBLK : (mb + 1) * SBLK],
                                    start=(h == 0),
                                    stop=(h == HPC - 1),
                                )
                        for mi in range(2):
                            mb = mp * 2 + mi
                            y_sb = ypool.tile([P, SBLK], DT.float32, tag="y")
                            nc.vector.tensor_copy(y_sb[:], pyms[mi][:])
                            nc.sync.dma_start(
                                out=y_d[srow : srow + P, mb * SBLK : (mb + 1) * SBLK],
                                in_=y_sb[:],
                            )

            for sq in range(SQ):
                x_chunk = xpool.tile([P, KT, SBLK], WDT, tag="xq")
                nc.sync.dma_start(out=x_chunk[:], in_=x_d[sq])
                do_qk(sq, x_chunk, wq_d, qt_sb)
                do_qk(sq, x_chunk, wk_d, kt_sb)
                do_v(sq, x_chunk)
                aotg = aotpool.tile([P, HPC, SBLK], WDT, tag="aot")
                for h in range(HPC):
                    do_attn_head(sq, h, aotg)
                do_y(sq, aotg)

    nc.compile()
    return nc


def _pack_inputs(hidden_states, Wq, Wk, Wv, Wo):
    """Per-core input dicts. Core c = b*4 + hg."""
    scale = 1.0 / math.sqrt(HD)
    wnp = ml_dtypes.bfloat16 if WDT_NAME == "bf16" else np.float32

    # RoPE tables, transposed layout [d, s], sign folded into sin.
    inv_freq = (1.0 / (ROPE_THETA ** (np.arange(0, HD, 2) / HD))).astype(np.float64)
    freqs = np.arange(S, dtype=np.float64)[:, None] * inv_freq[None, :]  # [S, 64]
    cos_h = np.ascontiguousarray(np.cos(freqs).T.astype(np.float32))  # [64, S]
    sin_h = np.ascontiguousarray(np.sin(freqs).T.astype(np.float32))  # [64, S]

    in_maps = []
    for c in range(8):
        b, hg = c // NHG, c % NHG
        hs = np.ascontiguousarray(hidden_states[b])  # [S, H]
        x_packed = np.ascontiguousarray(
            hs.reshape(SQ, SBLK, KT, P).transpose(0, 3, 2, 1)
        )  # [sq, Ph, kt, s]

        def w_cols(Wmat, sc=1.0):
            A = (Wmat[hg * OC : (hg + 1) * OC, :] * sc).astype(np.float32)  # [o, h]
            return np.ascontiguousarray(
                A.T.reshape(KT, P, HPC, P).transpose(2, 1, 0, 3)
            )  # [t, Ph, kt, o]

        wq_p = w_cols(Wq, scale)
        wk_p = w_cols(Wk)
        wv_p = np.ascontiguousarray(
            Wv[hg * OC : (hg + 1) * OC, :].T.reshape(KT, P, OC)
        )  # [kt, Ph, o]
        wo_p = np.ascontiguousarray(
            Wo[:, hg * OC : (hg + 1) * OC].T.reshape(HPC, P, H).transpose(1, 0, 2)
        )  # [Po, h, m]

        in_maps.append(
            {
                "x": x_packed.astype(wnp),
                "wq": wq_p.astype(wnp),
                "wk": wk_p.astype(wnp),
                "wv": wv_p.astype(wnp),
                "wo": wo_p.astype(wnp),
                "cos": cos_h,
                "sin": sin_h,
            }
        )
    return in_maps


def _get_nc():
    if "nc" not in _CACHE:
        _CACHE["nc"] = _build_nc()
    return _CACHE["nc"]


def kernel(hidden_states, Wq, Wk, Wv, Wo, attention_mask=None, **_ignored):
    hidden_states = np.asarray(hidden_states, dtype=np.float32)
    Wq = np.asarray(Wq, dtype=np.float32)
    Wk = np.asarray(Wk, dtype=np.float32)
    Wv = np.asarray(Wv, dtype=np.float32)
    Wo = np.asarray(Wo, dtype=np.float32)

    nc = _get_nc()
    in_maps = _pack_inputs(hidden_states, Wq, Wk, Wv, Wo)

    trace = bool(os.environ.get("KERNEL_TRACE"))
    kwargs = {}
    if trace:
        import types

        try:
            import antenv.axon_hooks  # noqa: F401
        except ImportError:
            from trn_agent_boot.trn_boot import _ntff_profile_via_ctypes

            hook = _ntff_profile_via_ctypes("/opt/axon/libaxon_pjrt.so")
            m = types.ModuleType("antenv.axon_hooks")
            m.get_axon_ntff_profile_hook = lambda: hook
            sys.modules["antenv.axon_hooks"] = m
        from concourse import bass_utils as _bu

        _bu.upload_artifacts = lambda tmpdir: "local://" + tmpdir
        kwargs["trace"] = True

    res = run_bass_kernel_spmd(nc, in_maps, list(range(8)), **kwargs)
    _CACHE["last_exec_time_ns"] = res.exec_time_ns

    out = np.empty((B, S, H), dtype=np.float32)
    for b in range(B):
        acc = res.results[b * NHG + 0]["y"].astype(np.float32)
        for hg in range(1, NHG):
            acc = acc + res.results[b * NHG + hg]["y"]
        out[b] = acc
    return out
